# revision 9
# baseline (speedup 1.0000x reference)
"""GGNN (CFGEncoder) message-passing kernel for Trainium2, 8 NeuronCores.

Data-parallel over batch (B=16 -> 2 items/core x 8 cores), no collectives.

Precision scheme ("f16x2-scaled"): the big [N,NE]@[NE,D] message matmuls
run as three f16 products with an error-free-ish split
    A = Ah + Al/2048,  s = sh + sl/2048   (Ah=f16(A), Al=f16((A-Ah)*2048))
    a = Ah@sh  +  (Ah@sl + Al@sh) / 2048
which carries ~2^-22 operand precision at f16 streaming rate (1 cyc/row),
with the two edge directions running concurrently in the two column
halves of the PE array (dst psum partitions 0:64 / 64:128). The scaled
products accumulate in a separate PSUM tile, merged with a *2^-11
scalar_tensor_tensor at copy-out. Everything else (per-edge-type
transforms, GRU gates, output head) runs in exact fp32.

A^T (both directions) is materialized once per item via exact fp32
PE transpose-mode matmuls; the hi/lo f16 split happens during the
PSUM->SBUF copy. All compute is done in a transposed domain (feature
dim on partitions) so gate biases fuse into per-partition ACT biases
and the edge-type biases fold into an augmented ones-row matmul.
"""

import os
import sys

for _p in ("/opt/trn_rl_repo", "/root/.axon_site/_ro/trn_rl_repo"):
    if _p not in sys.path and os.path.isdir(_p):
        sys.path.append(_p)

import numpy as np

import concourse.bass as bass
import concourse.tile as tile
from concourse import bacc, mybir
from concourse.masks import make_identity

B, N, E, D = 16, 768, 3, 64
NE = N * E                    # 2304
N_STEPS = 5
N_CORES = 8
PER_CORE = B // N_CORES       # 2
NCH = N // 128                # 6 n-chunks
F32 = mybir.dt.float32
F16 = mybir.dt.float16
SC = 2048.0                   # lo-part scale (2^11)
AF = mybir.ActivationFunctionType
DEBUG_DUMP = bool(int(os.environ.get("GGNN_DEBUG_DUMP", "0")))

W_NAMES = ("W_in", "b_in", "W_out", "b_out",
           "W_r", "b_r", "W_z", "b_z", "W_h", "b_h", "W_o", "b_o")


def _build_program():
    nc = bacc.Bacc(
        "TRN2", target_bir_lowering=False, debug=False, num_devices=N_CORES
    )
    d = {}
    d["A"] = nc.dram_tensor("A", [PER_CORE, N, 2 * NE], F32,
                            kind="ExternalInput").ap()
    d["prop_state"] = nc.dram_tensor("prop_state", [PER_CORE, N, D], F32,
                                     kind="ExternalInput").ap()
    d["W_in"] = nc.dram_tensor("W_in", [E, D, D], F32, kind="ExternalInput").ap()
    d["b_in"] = nc.dram_tensor("b_in", [E, D], F32, kind="ExternalInput").ap()
    d["W_out"] = nc.dram_tensor("W_out", [E, D, D], F32, kind="ExternalInput").ap()
    d["b_out"] = nc.dram_tensor("b_out", [E, D], F32, kind="ExternalInput").ap()
    for nm in ("W_r", "W_z", "W_h"):
        d[nm] = nc.dram_tensor(nm, [D, 3 * D], F32, kind="ExternalInput").ap()
    for nm in ("b_r", "b_z", "b_h", "b_o"):
        d[nm] = nc.dram_tensor(nm, [D], F32, kind="ExternalInput").ap()
    d["W_o"] = nc.dram_tensor("W_o", [D, 2 * D], F32, kind="ExternalInput").ap()
    d["out"] = nc.dram_tensor("out", [PER_CORE, N, D], F32,
                              kind="ExternalOutput").ap()
    dbg = {}
    if DEBUG_DUMP:
        for nm, shape in (("dbg_ain", [64, N]), ("dbg_aout", [64, N]),
                          ("dbg_hT1", [65, N])):
            dbg[nm] = nc.dram_tensor(nm, shape, F32, kind="ExternalOutput").ap()

    with tile.TileContext(nc) as tc:
        _emit(nc, tc, d, dbg)
    nc.compile()
    return nc


def _emit(nc, tc, d, dbg):
    from contextlib import ExitStack

    ctx = ExitStack()
    const = ctx.enter_context(tc.tile_pool(name="const", bufs=1))
    stage = ctx.enter_context(tc.tile_pool(name="stage", bufs=1))
    atp = ctx.enter_context(tc.tile_pool(name="at", bufs=1))
    work = ctx.enter_context(tc.tile_pool(name="work", bufs=1))
    ps_tr = ctx.enter_context(tc.tile_pool(name="ps_tr", bufs=1, space="PSUM"))
    ps_s = ctx.enter_context(tc.tile_pool(name="ps_s", bufs=1, space="PSUM"))
    ps_am = ctx.enter_context(tc.tile_pool(name="ps_am", bufs=1, space="PSUM"))
    ps_as = ctx.enter_context(tc.tile_pool(name="ps_as", bufs=1, space="PSUM"))
    ps_g = ctx.enter_context(tc.tile_pool(name="ps_g", bufs=1, space="PSUM"))

    # ---------------- constants / weights ----------------
    ident = const.tile([128, 128], F32, tag="ident")
    make_identity(nc, ident[:, :])

    # Augmented per-edge-type transform weights: rows 0:64 = W^T blocks,
    # row 64 = biases. Column layout: dir*192 + e*64 + d_out.
    w_s = const.tile([65, 2 * E * D], F32, tag="w_s")
    for di, (wn, bn) in enumerate((("W_in", "b_in"), ("W_out", "b_out"))):
        for e in range(E):
            c0 = di * E * D + e * D
            nc.sync.dma_start(w_s[0:64, c0:c0 + D], d[wn][e].transpose([1, 0]))
            nc.sync.dma_start(w_s[64:65, c0:c0 + D], d[bn][e:e + 1, :])

    # Gate weights, transposed per 64-chunk of the 3D contraction axis.
    # w_rz columns: c*128 + [r block 64 | z block 64], c in {a_in, a_out, h}
    w_rz = const.tile([64, 3 * 128], F32, tag="w_rz")
    w_h = const.tile([64, 3 * 64], F32, tag="w_h")
    for c in range(3):
        cs = slice(c * 64, (c + 1) * 64)
        nc.sync.dma_start(w_rz[:, c * 128:c * 128 + 64],
                          d["W_r"][:, cs].transpose([1, 0]))
        nc.sync.dma_start(w_rz[:, c * 128 + 64:c * 128 + 128],
                          d["W_z"][:, cs].transpose([1, 0]))
        nc.sync.dma_start(w_h[:, cs], d["W_h"][:, cs].transpose([1, 0]))
    w_o_h = const.tile([64, 64], F32, tag="w_o_h")
    w_o_h0 = const.tile([64, 64], F32, tag="w_o_h0")
    nc.sync.dma_start(w_o_h[:, :], d["W_o"][:, 0:64].transpose([1, 0]))
    nc.sync.dma_start(w_o_h0[:, :], d["W_o"][:, 64:128].transpose([1, 0]))

    cols = {}
    for nm in ("b_r", "b_z", "b_h", "b_o"):
        col = const.tile([64, 1], F32, tag=f"{nm}_col", name=f"{nm}_col")
        nc.sync.dma_start(col[:, 0:1], d[nm].unsqueeze(1))
        cols[nm] = col

    consts = dict(ident=ident, w_s=w_s, w_rz=w_rz, w_h=w_h,
                  w_o_h=w_o_h, w_o_h0=w_o_h0, cols=cols)
    pools = dict(stage=stage, atp=atp, work=work, ps_tr=ps_tr, ps_s=ps_s,
                 ps_am=ps_am, ps_as=ps_as, ps_g=ps_g)
    for it in range(PER_CORE):
        _emit_item(nc, tc, it, d, dbg, consts, pools)
    ctx.close()


def _emit_item(nc, tc, it, d, dbg, consts, pools):
    ident = consts["ident"]
    cols = consts["cols"]
    stage, atp, work = pools["stage"], pools["atp"], pools["work"]
    ps_tr, ps_s = pools["ps_tr"], pools["ps_s"]
    ps_am, ps_as, ps_g = pools["ps_am"], pools["ps_as"], pools["ps_g"]

    # ---------------- load + transpose prop_state ----------------
    hT = work.tile([65, N], F32, tag="hT")    # rows 0:64 h^T, row 64 ones
    h0T = work.tile([64, N], F32, tag="h0T")
    nc.gpsimd.memset(hT[64:65, :], 1.0)

    pstage = stage.tile([128, NCH * D], F32, tag="pstage")
    nc.sync.dma_start(pstage[:, :],
                      d["prop_state"][it].rearrange("(c p) d -> p c d", p=128))
    for c in range(NCH):
        ptr = ps_tr.tile([128, 384], F32, tag="tr")
        nc.tensor.transpose(ptr[0:64, 0:128], pstage[:, c * D:(c + 1) * D],
                            ident[:, :])
        nc.scalar.copy(hT[0:64, c * 128:(c + 1) * 128], ptr[0:64, 0:128])
        nc.vector.tensor_copy(h0T[:, c * 128:(c + 1) * 128], ptr[0:64, 0:128])

    # ---------------- load + transpose + f16-split A ----------------
    # at_hi/at_lo organized as 6 tensors each; tensor g holds k-tiles
    # j in [6g, 6g+6): at_*[g][p, 768*i + n] covers A[n, 128*(6g+i)+p].
    # at_lo holds (A - f16(A)) * 2048.
    at_hi = [atp.tile([128, NCH * N], F16, tag=f"ath{g}", name=f"ath{g}_{it}")
             for g in range(6)]
    at_lo = [atp.tile([128, NCH * N], F16, tag=f"atl{g}", name=f"atl{g}_{it}")
             for g in range(6)]
    for c in range(NCH):            # n-chunk (A rows 128c .. 128c+128)
        for kh in range(2):         # halves of the 4608 k columns
            ast = stage.tile([128, NE], F32, tag="astage")
            nc.sync.dma_start(
                ast[:, :], d["A"][it, c * 128:(c + 1) * 128,
                                  kh * NE:(kh + 1) * NE])
            for grp in range(6):    # groups of 3 k-tiles -> one psum bank
                ptr = ps_tr.tile([128, 384], F32, tag="tr")
                for jl in range(3):
                    jj = grp * 3 + jl
                    nc.tensor.transpose(
                        ptr[:, jl * 128:(jl + 1) * 128],
                        ast[:, jj * 128:(jj + 1) * 128], ident[:, :])
                j0 = kh * 18 + grp * 3
                g, i0 = divmod(j0, 6)
                sel = (slice(None), slice(i0, i0 + 3),
                       slice(c * 128, (c + 1) * 128))
                hi_dst = at_hi[g].rearrange("p (i n) -> p i n", i=NCH)[sel]
                lo_dst = at_lo[g].rearrange("p (i n) -> p i n", i=NCH)[sel]
                src3 = ptr[:, :].rearrange("p (i n) -> p i n", i=3)
                nc.vector.tensor_copy(hi_dst, src3)          # f16 round (hi)
                nc.vector.tensor_sub(src3, src3, hi_dst)     # psum -= hi
                nc.scalar.activation(lo_dst, src3, AF.Copy, scale=SC)

    at_all = dict(hi=at_hi, lo=at_lo)

    def at_sl(which, j, lo, sz):
        g, i = divmod(j, 6)
        base = i * N
        return at_all[which][g][:, base + lo:base + lo + sz]

    w_s, w_rz, w_h = consts["w_s"], consts["w_rz"], consts["w_h"]
    w_o_h, w_o_h0 = consts["w_o_h"], consts["w_o_h0"]

    # ---------------- propagation steps ----------------
    for step in range(N_STEPS):
        # s = h @ W^T + b (both directions, all edge types) in fp32,
        # split into f16 hi/lo at PSUM copy-out.
        s_hi, s_lo = [], []
        for c in range(NCH):
            ps = ps_s.tile([128, 384], F32, tag="s")
            nc.tensor.matmul(ps[:, :], hT[:, c * 128:(c + 1) * 128],
                             w_s[:, :], start=True, stop=True)
            sh = work.tile([128, 384], F16, tag=f"sh{c}", name=f"sh{c}")
            sl = work.tile([128, 384], F16, tag=f"sl{c}", name=f"sl{c}")
            nc.vector.tensor_copy(sh[:, :], ps[:, :])
            nc.vector.tensor_sub(ps[:, :], ps[:, :], sh[:, :])
            nc.scalar.activation(sl[:, :], ps[:, :], AF.Copy, scale=SC)
            s_hi.append(sh)
            s_lo.append(sl)

        # message matmuls: three f16 products, two directions concurrent
        # in the two column halves (psum partitions 0:64 / 64:128).
        pm = ps_am.tile([128, N], F32, tag="am")    # main: Ah@sh
        pscl = ps_as.tile([128, N], F32, tag="as")  # scaled: Ah@sl + Al@sh
        first = True
        for c in range(NCH):
            for e in range(E):
                j = e * NCH + c
                last = (c == NCH - 1 and e == E - 1)
                sin = slice(e * D, (e + 1) * D)
                sout = slice(192 + e * D, 192 + (e + 1) * D)
                for lo, sz in ((0, 512), (512, 256)):
                    psl = slice(lo, lo + sz)
                    nc.tensor.matmul(pm[0:64, psl], s_hi[c][:, sin],
                                     at_sl("hi", j, lo, sz),
                                     start=first, stop=last)
                    nc.tensor.matmul(pm[64:128, psl], s_hi[c][:, sout],
                                     at_sl("hi", 18 + j, lo, sz),
                                     start=first, stop=last)
                    nc.tensor.matmul(pscl[0:64, psl], s_lo[c][:, sin],
                                     at_sl("hi", j, lo, sz),
                                     start=first, stop=False)
                    nc.tensor.matmul(pscl[64:128, psl], s_lo[c][:, sout],
                                     at_sl("hi", 18 + j, lo, sz),
                                     start=first, stop=False)
                    nc.tensor.matmul(pscl[0:64, psl], s_hi[c][:, sin],
                                     at_sl("lo", j, lo, sz),
                                     start=False, stop=last)
                    nc.tensor.matmul(pscl[64:128, psl], s_hi[c][:, sout],
                                     at_sl("lo", 18 + j, lo, sz),
                                     start=False, stop=last)
                first = False

        # merge: a = main + scaled / 2048
        a_in = work.tile([64, N], F32, tag="a_in")
        a_out = work.tile([64, N], F32, tag="a_out")
        nc.scalar.copy(a_in[:, :], pm[0:64, :])
        nc.vector.scalar_tensor_tensor(
            a_in[:, :], pscl[0:64, :], 1.0 / SC, a_in[:, :],
            op0=mybir.AluOpType.mult, op1=mybir.AluOpType.add)
        nc.scalar.copy(a_out[:, :], pm[64:128, :])
        nc.vector.scalar_tensor_tensor(
            a_out[:, :], pscl[64:128, :], 1.0 / SC, a_out[:, :],
            op0=mybir.AluOpType.mult, op1=mybir.AluOpType.add)

        if DEBUG_DUMP and it == 0 and step == 0:
            nc.sync.dma_start(dbg["dbg_ain"][:, :], a_in[:, :])
            nc.sync.dma_start(dbg["dbg_aout"][:, :], a_out[:, :])

        # gates r, z (fused matmul, fp32): psum rows 0:64 = r, 64:128 = z
        pg = ps_g.tile([128, N], F32, tag="g")
        for ci, csrc in enumerate((a_in, a_out, hT)):
            src = csrc[0:64, :] if ci == 2 else csrc[:, :]
            for lo, sz in ((0, 512), (512, 256)):
                nc.tensor.matmul(pg[:, lo:lo + sz],
                                 w_rz[:, ci * 128:(ci + 1) * 128],
                                 src[:, lo:lo + sz],
                                 start=(ci == 0), stop=(ci == 2))
        r_sb = work.tile([64, N], F32, tag="r_sb")
        nc.scalar.activation(r_sb[:, :], pg[0:64, :], AF.Sigmoid,
                             bias=cols["b_r"][:, 0:1])
        z_sb = work.tile([64, N], F32, tag="z_sb")
        nc.scalar.activation(z_sb[:, :], pg[64:128, :], AF.Sigmoid,
                             bias=cols["b_z"][:, 0:1])

        rh = work.tile([64, N], F32, tag="rh")
        nc.vector.tensor_mul(rh[:, :], r_sb[:, :], hT[0:64, :])

        # h_hat pre-activation (fp32)
        pg2 = ps_g.tile([128, N], F32, tag="g")
        for ci, csrc in enumerate((a_in, a_out, rh)):
            for lo, sz in ((0, 512), (512, 256)):
                nc.tensor.matmul(pg2[0:64, lo:lo + sz],
                                 w_h[:, ci * D:(ci + 1) * D],
                                 csrc[:, lo:lo + sz],
                                 start=(ci == 0), stop=(ci == 2))
        hh = work.tile([64, N], F32, tag="hh")
        nc.scalar.activation(hh[:, :], pg2[0:64, :], AF.Tanh,
                             bias=cols["b_h"][:, 0:1])

        # h <- h + z * (h_hat - h)
        d1 = work.tile([64, N], F32, tag="d1")
        nc.vector.tensor_sub(d1[:, :], hh[:, :], hT[0:64, :])
        d2 = work.tile([64, N], F32, tag="d2")
        nc.vector.tensor_mul(d2[:, :], d1[:, :], z_sb[:, :])
        nc.vector.tensor_add(hT[0:64, :], hT[0:64, :], d2[:, :])
        if DEBUG_DUMP and it == 0 and step == 0:
            nc.sync.dma_start(dbg["dbg_hT1"][:, :], hT[:, :])

    # ---------------- output head (fp32) ----------------
    pg3 = ps_g.tile([128, N], F32, tag="g")
    for lo, sz in ((0, 512), (512, 256)):
        nc.tensor.matmul(pg3[0:64, lo:lo + sz], w_o_h[:, :],
                         hT[0:64, lo:lo + sz], start=True, stop=False)
        nc.tensor.matmul(pg3[0:64, lo:lo + sz], w_o_h0[:, :],
                         h0T[:, lo:lo + sz], start=False, stop=True)
    oT = work.tile([64, N], F32, tag="oT")
    nc.scalar.activation(oT[:, :], pg3[0:64, :], AF.Tanh,
                         bias=cols["b_o"][:, 0:1])

    o_sb = work.tile([128, NCH * D], F32, tag="o_sb")
    for c in range(NCH):
        ptr = ps_tr.tile([128, 384], F32, tag="tr")
        nc.tensor.transpose(ptr[:, 0:64], oT[:, c * 128:(c + 1) * 128],
                            ident[0:64, 0:64])
        nc.scalar.copy(o_sb[:, c * D:(c + 1) * D], ptr[:, 0:64])
    nc.sync.dma_start(d["out"][it].rearrange("(c p) d -> p c d", p=128),
                      o_sb[:, :])


_CACHE = {}


def _get_program():
    if "nc" not in _CACHE:
        _CACHE["nc"] = _build_program()
    return _CACHE["nc"]


def kernel(**inputs):
    nc = _get_program()
    from concourse.bass_utils import run_bass_kernel_spmd

    A = np.ascontiguousarray(np.asarray(inputs["A"], dtype=np.float32))
    prop = np.ascontiguousarray(np.asarray(inputs["prop_state"],
                                           dtype=np.float32))
    shared = {nm: np.ascontiguousarray(np.asarray(inputs[nm], dtype=np.float32))
              for nm in W_NAMES}
    in_maps = []
    for c in range(N_CORES):
        sl = slice(c * PER_CORE, (c + 1) * PER_CORE)
        in_maps.append({"A": A[sl], "prop_state": prop[sl], **shared})

    res = run_bass_kernel_spmd(nc, in_maps, core_ids=list(range(N_CORES)))
    out = np.concatenate([res.results[c]["out"] for c in range(N_CORES)],
                         axis=0)
    return out.astype(np.float32)


if __name__ == "__main__":
    rng = np.random.default_rng(0)
    inputs = {
        "prop_state": rng.standard_normal((B, N, D), dtype=np.float32),
        "annotation": rng.standard_normal((B, N, D), dtype=np.float32),
        "A": rng.random((B, N, 2 * NE), dtype=np.float32),
        "node_mask": np.ones((B, N), dtype=bool),
        "W_in": rng.standard_normal((E, D, D), dtype=np.float32) * 0.05,
        "b_in": rng.standard_normal((E, D), dtype=np.float32) * 0.05,
        "W_out": rng.standard_normal((E, D, D), dtype=np.float32) * 0.05,
        "b_out": rng.standard_normal((E, D), dtype=np.float32) * 0.05,
        "W_r": rng.standard_normal((D, 3 * D), dtype=np.float32) * 0.05,
        "b_r": rng.standard_normal((D,), dtype=np.float32) * 0.05,
        "W_z": rng.standard_normal((D, 3 * D), dtype=np.float32) * 0.05,
        "b_z": rng.standard_normal((D,), dtype=np.float32) * 0.05,
        "W_h": rng.standard_normal((D, 3 * D), dtype=np.float32) * 0.05,
        "b_h": rng.standard_normal((D,), dtype=np.float32) * 0.05,
        "W_o": rng.standard_normal((D, 2 * D), dtype=np.float32) * 0.05,
        "b_o": rng.standard_normal((D,), dtype=np.float32) * 0.05,
    }
    out = kernel(**inputs)
    print("out", out.shape, out.dtype, float(np.abs(out).max()))


# revision 11
# speedup vs baseline: 59.3434x; 59.3434x over previous
"""GGNN (CFGEncoder) message-passing kernel for Trainium2, 8 NeuronCores.

Data-parallel over batch (B=16 -> 2 items/core x 8 cores), no collectives.

Precision scheme ("f16x2-scaled"): the big [N,NE]@[NE,D] message matmuls
run as three f16 products with an error-free-ish split
    A = Ah + Al/2048,  s = sh + sl/2048   (Ah=f16(A), Al=f16((A-Ah)*2048))
    a = Ah@sh  +  (Ah@sl + Al@sh) / 2048
which carries ~2^-22 operand precision at f16 streaming rate (1 cyc/row),
with the two edge directions running concurrently in the two column
halves of the PE array (dst psum partitions 0:64 / 64:128). The scaled
products accumulate in a separate PSUM tile, merged with a *2^-11
scalar_tensor_tensor at copy-out. Everything else (per-edge-type
transforms, GRU gates, output head) runs in exact fp32.

A^T (both directions) is materialized once per item via exact fp32
PE transpose-mode matmuls; the hi/lo f16 split happens during the
PSUM->SBUF copy. All compute is done in a transposed domain (feature
dim on partitions) so gate biases fuse into per-partition ACT biases
and the edge-type biases fold into an augmented ones-row matmul.
"""

import os
import sys

for _p in ("/opt/trn_rl_repo", "/root/.axon_site/_ro/trn_rl_repo"):
    if _p not in sys.path and os.path.isdir(_p):
        sys.path.append(_p)

import numpy as np

import concourse.bass as bass
import concourse.tile as tile
from concourse import bacc, mybir
from concourse.masks import make_identity

B, N, E, D = 16, 768, 3, 64
NE = N * E                    # 2304
N_STEPS = 5
N_CORES = 8
PER_CORE = B // N_CORES       # 2
NCH = N // 128                # 6 n-chunks
F32 = mybir.dt.float32
F16 = mybir.dt.float16
SC = 2048.0                   # lo-part scale (2^11)
AF = mybir.ActivationFunctionType
DEBUG_DUMP = bool(int(os.environ.get("GGNN_DEBUG_DUMP", "0")))

W_NAMES = ("W_in", "b_in", "W_out", "b_out",
           "W_r", "b_r", "W_z", "b_z", "W_h", "b_h", "W_o", "b_o")


def _build_program():
    nc = bacc.Bacc(
        "TRN2", target_bir_lowering=False, debug=False, num_devices=N_CORES
    )
    d = {}
    d["A"] = nc.dram_tensor("A", [PER_CORE, N, 2 * NE], F32,
                            kind="ExternalInput").ap()
    d["prop_state"] = nc.dram_tensor("prop_state", [PER_CORE, N, D], F32,
                                     kind="ExternalInput").ap()
    d["W_in"] = nc.dram_tensor("W_in", [E, D, D], F32, kind="ExternalInput").ap()
    d["b_in"] = nc.dram_tensor("b_in", [E, D], F32, kind="ExternalInput").ap()
    d["W_out"] = nc.dram_tensor("W_out", [E, D, D], F32, kind="ExternalInput").ap()
    d["b_out"] = nc.dram_tensor("b_out", [E, D], F32, kind="ExternalInput").ap()
    for nm in ("W_r", "W_z", "W_h"):
        d[nm] = nc.dram_tensor(nm, [D, 3 * D], F32, kind="ExternalInput").ap()
    for nm in ("b_r", "b_z", "b_h", "b_o"):
        d[nm] = nc.dram_tensor(nm, [D], F32, kind="ExternalInput").ap()
    d["W_o"] = nc.dram_tensor("W_o", [D, 2 * D], F32, kind="ExternalInput").ap()
    d["out"] = nc.dram_tensor("out", [PER_CORE, N, D], F32,
                              kind="ExternalOutput").ap()
    dbg = {}
    if DEBUG_DUMP:
        for nm, shape in (("dbg_ain", [64, N]), ("dbg_aout", [64, N]),
                          ("dbg_hT1", [65, N])):
            dbg[nm] = nc.dram_tensor(nm, shape, F32, kind="ExternalOutput").ap()

    with tile.TileContext(nc) as tc:
        _emit(nc, tc, d, dbg)
    nc.compile()
    return nc


def _emit(nc, tc, d, dbg):
    from contextlib import ExitStack

    ctx = ExitStack()
    const = ctx.enter_context(tc.tile_pool(name="const", bufs=1))
    stage = ctx.enter_context(tc.tile_pool(name="stage", bufs=1))
    atp = ctx.enter_context(tc.tile_pool(name="at", bufs=1))
    work = ctx.enter_context(tc.tile_pool(name="work", bufs=1))
    ps_tr = ctx.enter_context(tc.tile_pool(name="ps_tr", bufs=1, space="PSUM"))
    ps_s = ctx.enter_context(tc.tile_pool(name="ps_s", bufs=1, space="PSUM"))
    ps_am = ctx.enter_context(tc.tile_pool(name="ps_am", bufs=1, space="PSUM"))
    ps_as = ctx.enter_context(tc.tile_pool(name="ps_as", bufs=1, space="PSUM"))
    ps_g = ctx.enter_context(tc.tile_pool(name="ps_g", bufs=1, space="PSUM"))

    # ---------------- constants / weights ----------------
    ident = const.tile([128, 128], F32, tag="ident")
    make_identity(nc, ident[:, :])

    # Augmented per-edge-type transform weights: rows 0:64 = W^T blocks,
    # row 64 = biases. Column layout: dir*192 + e*64 + d_out.
    w_s = const.tile([65, 2 * E * D], F32, tag="w_s")
    for di, (wn, bn) in enumerate((("W_in", "b_in"), ("W_out", "b_out"))):
        for e in range(E):
            c0 = di * E * D + e * D
            nc.sync.dma_start(w_s[0:64, c0:c0 + D], d[wn][e].transpose([1, 0]))
            nc.sync.dma_start(w_s[64:65, c0:c0 + D], d[bn][e:e + 1, :])

    # Gate weights, transposed per 64-chunk of the 3D contraction axis.
    # w_rz columns: c*128 + [r block 64 | z block 64], c in {a_in, a_out, h}
    w_rz = const.tile([64, 3 * 128], F32, tag="w_rz")
    w_h = const.tile([64, 3 * 64], F32, tag="w_h")
    for c in range(3):
        cs = slice(c * 64, (c + 1) * 64)
        nc.sync.dma_start(w_rz[:, c * 128:c * 128 + 64],
                          d["W_r"][:, cs].transpose([1, 0]))
        nc.sync.dma_start(w_rz[:, c * 128 + 64:c * 128 + 128],
                          d["W_z"][:, cs].transpose([1, 0]))
        nc.sync.dma_start(w_h[:, cs], d["W_h"][:, cs].transpose([1, 0]))
    w_o_h = const.tile([64, 64], F32, tag="w_o_h")
    w_o_h0 = const.tile([64, 64], F32, tag="w_o_h0")
    nc.sync.dma_start(w_o_h[:, :], d["W_o"][:, 0:64].transpose([1, 0]))
    nc.sync.dma_start(w_o_h0[:, :], d["W_o"][:, 64:128].transpose([1, 0]))

    cols = {}
    for nm in ("b_r", "b_z", "b_h", "b_o"):
        col = const.tile([64, 1], F32, tag=f"{nm}_col", name=f"{nm}_col")
        nc.sync.dma_start(col[:, 0:1], d[nm].unsqueeze(1))
        cols[nm] = col

    consts = dict(ident=ident, w_s=w_s, w_rz=w_rz, w_h=w_h,
                  w_o_h=w_o_h, w_o_h0=w_o_h0, cols=cols)
    pools = dict(stage=stage, atp=atp, work=work, ps_tr=ps_tr, ps_s=ps_s,
                 ps_am=ps_am, ps_as=ps_as, ps_g=ps_g)
    for it in range(PER_CORE):
        _emit_item(nc, tc, it, d, dbg, consts, pools)
    ctx.close()


def _emit_item(nc, tc, it, d, dbg, consts, pools):
    ident = consts["ident"]
    cols = consts["cols"]
    stage, atp, work = pools["stage"], pools["atp"], pools["work"]
    ps_tr, ps_s = pools["ps_tr"], pools["ps_s"]
    ps_am, ps_as, ps_g = pools["ps_am"], pools["ps_as"], pools["ps_g"]

    # ---------------- load + transpose prop_state ----------------
    hT = work.tile([65, N], F32, tag="hT")    # rows 0:64 h^T, row 64 ones
    h0T = work.tile([64, N], F32, tag="h0T")
    nc.gpsimd.memset(hT[64:65, :], 1.0)

    pstage = stage.tile([128, NCH * D], F32, tag="pstage")
    nc.sync.dma_start(pstage[:, :],
                      d["prop_state"][it].rearrange("(c p) d -> p c d", p=128))
    for c in range(NCH):
        ptr = ps_tr.tile([128, 384], F32, tag="tr")
        nc.tensor.transpose(ptr[0:64, 0:128], pstage[:, c * D:(c + 1) * D],
                            ident[:, :])
        nc.scalar.copy(hT[0:64, c * 128:(c + 1) * 128], ptr[0:64, 0:128])
        nc.vector.tensor_copy(h0T[:, c * 128:(c + 1) * 128], ptr[0:64, 0:128])

    # ---------------- load + transpose + f16-split A ----------------
    # at_hi/at_lo organized as 6 tensors each; tensor g holds k-tiles
    # j in [6g, 6g+6): at_*[g][p, 768*i + n] covers A[n, 128*(6g+i)+p].
    # at_lo holds (A - f16(A)) * 2048.
    at_hi = [atp.tile([128, NCH * N], F16, tag=f"ath{g}", name=f"ath{g}_{it}")
             for g in range(6)]
    at_lo = [atp.tile([128, NCH * N], F16, tag=f"atl{g}", name=f"atl{g}_{it}")
             for g in range(6)]
    for c in range(NCH):            # n-chunk (A rows 128c .. 128c+128)
        for kh in range(2):         # halves of the 4608 k columns
            ast = stage.tile([128, NE], F32, tag="astage")
            nc.sync.dma_start(
                ast[:, :], d["A"][it, c * 128:(c + 1) * 128,
                                  kh * NE:(kh + 1) * NE])
            for grp in range(6):    # groups of 3 k-tiles -> one psum bank
                ptr = ps_tr.tile([128, 384], F32, tag="tr")
                for jl in range(3):
                    jj = grp * 3 + jl
                    nc.tensor.transpose(
                        ptr[:, jl * 128:(jl + 1) * 128],
                        ast[:, jj * 128:(jj + 1) * 128], ident[:, :])
                j0 = kh * 18 + grp * 3
                g, i0 = divmod(j0, 6)
                sel = (slice(None), slice(i0, i0 + 3),
                       slice(c * 128, (c + 1) * 128))
                hi_dst = at_hi[g].rearrange("p (i n) -> p i n", i=NCH)[sel]
                lo_dst = at_lo[g].rearrange("p (i n) -> p i n", i=NCH)[sel]
                src3 = ptr[:, :].rearrange("p (i n) -> p i n", i=3)
                nc.vector.tensor_copy(hi_dst, src3)          # f16 round (hi)
                nc.vector.tensor_sub(src3, src3, hi_dst)     # psum -= hi
                nc.scalar.activation(lo_dst, src3, AF.Copy, scale=SC)

    at_all = dict(hi=at_hi, lo=at_lo)

    def at_sl(which, j, lo, sz):
        g, i = divmod(j, 6)
        base = i * N
        return at_all[which][g][:, base + lo:base + lo + sz]

    w_s, w_rz, w_h = consts["w_s"], consts["w_rz"], consts["w_h"]
    w_o_h, w_o_h0 = consts["w_o_h"], consts["w_o_h0"]

    # ---------------- propagation steps ----------------
    for step in range(N_STEPS):
        # s = h @ W^T + b (both directions, all edge types) in fp32,
        # split into f16 hi/lo at PSUM copy-out.
        s_hi, s_lo = [], []
        for c in range(NCH):
            ps = ps_s.tile([128, 384], F32, tag="s")
            nc.tensor.matmul(ps[:, :], hT[:, c * 128:(c + 1) * 128],
                             w_s[:, :], start=True, stop=True)
            sh = work.tile([128, 384], F16, tag=f"sh{c}", name=f"sh{c}")
            sl = work.tile([128, 384], F16, tag=f"sl{c}", name=f"sl{c}")
            nc.vector.tensor_copy(sh[:, :], ps[:, :])
            nc.vector.tensor_sub(ps[:, :], ps[:, :], sh[:, :])
            nc.scalar.activation(sl[:, :], ps[:, :], AF.Copy, scale=SC)
            s_hi.append(sh)
            s_lo.append(sl)

        # message matmuls: three f16 products, two directions concurrent
        # in the two column halves (psum partitions 0:64 / 64:128).
        pm = ps_am.tile([128, N], F32, tag="am")    # main: Ah@sh
        pscl = ps_as.tile([128, N], F32, tag="as")  # scaled: Ah@sl + Al@sh
        first = True
        for c in range(NCH):
            for e in range(E):
                j = e * NCH + c
                last = (c == NCH - 1 and e == E - 1)
                sin = slice(e * D, (e + 1) * D)
                sout = slice(192 + e * D, 192 + (e + 1) * D)
                for lo, sz in ((0, 512), (512, 256)):
                    psl = slice(lo, lo + sz)
                    nc.tensor.matmul(pm[0:64, psl], s_hi[c][:, sin],
                                     at_sl("hi", j, lo, sz),
                                     start=first, stop=last)
                    nc.tensor.matmul(pm[64:128, psl], s_hi[c][:, sout],
                                     at_sl("hi", 18 + j, lo, sz),
                                     start=first, stop=last)
                    nc.tensor.matmul(pscl[0:64, psl], s_lo[c][:, sin],
                                     at_sl("hi", j, lo, sz),
                                     start=first, stop=False)
                    nc.tensor.matmul(pscl[64:128, psl], s_lo[c][:, sout],
                                     at_sl("hi", 18 + j, lo, sz),
                                     start=first, stop=False)
                    nc.tensor.matmul(pscl[0:64, psl], s_hi[c][:, sin],
                                     at_sl("lo", j, lo, sz),
                                     start=False, stop=last)
                    nc.tensor.matmul(pscl[64:128, psl], s_hi[c][:, sout],
                                     at_sl("lo", 18 + j, lo, sz),
                                     start=False, stop=last)
                first = False

        # merge: a = main + scaled / 2048
        a_in = work.tile([64, N], F32, tag="a_in")
        a_out = work.tile([64, N], F32, tag="a_out")
        nc.scalar.copy(a_in[:, :], pm[0:64, :])
        nc.vector.scalar_tensor_tensor(
            a_in[:, :], pscl[0:64, :], 1.0 / SC, a_in[:, :],
            op0=mybir.AluOpType.mult, op1=mybir.AluOpType.add)
        nc.scalar.copy(a_out[:, :], pm[64:128, :])
        nc.vector.scalar_tensor_tensor(
            a_out[:, :], pscl[64:128, :], 1.0 / SC, a_out[:, :],
            op0=mybir.AluOpType.mult, op1=mybir.AluOpType.add)

        if DEBUG_DUMP and it == 0 and step == 0:
            nc.sync.dma_start(dbg["dbg_ain"][:, :], a_in[:, :])
            nc.sync.dma_start(dbg["dbg_aout"][:, :], a_out[:, :])

        # gates r, z (fused matmul, fp32): psum rows 0:64 = r, 64:128 = z
        pg = ps_g.tile([128, N], F32, tag="g")
        for ci, csrc in enumerate((a_in, a_out, hT)):
            src = csrc[0:64, :] if ci == 2 else csrc[:, :]
            for lo, sz in ((0, 512), (512, 256)):
                nc.tensor.matmul(pg[:, lo:lo + sz],
                                 w_rz[:, ci * 128:(ci + 1) * 128],
                                 src[:, lo:lo + sz],
                                 start=(ci == 0), stop=(ci == 2))
        r_sb = work.tile([64, N], F32, tag="r_sb")
        nc.scalar.activation(r_sb[:, :], pg[0:64, :], AF.Sigmoid,
                             bias=cols["b_r"][:, 0:1])
        z_sb = work.tile([64, N], F32, tag="z_sb")
        nc.scalar.activation(z_sb[:, :], pg[64:128, :], AF.Sigmoid,
                             bias=cols["b_z"][:, 0:1])

        rh = work.tile([64, N], F32, tag="rh")
        nc.vector.tensor_mul(rh[:, :], r_sb[:, :], hT[0:64, :])

        # h_hat pre-activation (fp32)
        pg2 = ps_g.tile([128, N], F32, tag="g")
        for ci, csrc in enumerate((a_in, a_out, rh)):
            for lo, sz in ((0, 512), (512, 256)):
                nc.tensor.matmul(pg2[0:64, lo:lo + sz],
                                 w_h[:, ci * D:(ci + 1) * D],
                                 csrc[:, lo:lo + sz],
                                 start=(ci == 0), stop=(ci == 2))
        hh = work.tile([64, N], F32, tag="hh")
        nc.scalar.activation(hh[:, :], pg2[0:64, :], AF.Tanh,
                             bias=cols["b_h"][:, 0:1])

        # h <- h + z * (h_hat - h)
        d1 = work.tile([64, N], F32, tag="d1")
        nc.vector.tensor_sub(d1[:, :], hh[:, :], hT[0:64, :])
        d2 = work.tile([64, N], F32, tag="d2")
        nc.vector.tensor_mul(d2[:, :], d1[:, :], z_sb[:, :])
        nc.vector.tensor_add(hT[0:64, :], hT[0:64, :], d2[:, :])
        if DEBUG_DUMP and it == 0 and step == 0:
            nc.sync.dma_start(dbg["dbg_hT1"][:, :], hT[:, :])

    # ---------------- output head (fp32) ----------------
    pg3 = ps_g.tile([128, N], F32, tag="g")
    for lo, sz in ((0, 512), (512, 256)):
        nc.tensor.matmul(pg3[0:64, lo:lo + sz], w_o_h[:, :],
                         hT[0:64, lo:lo + sz], start=True, stop=False)
        nc.tensor.matmul(pg3[0:64, lo:lo + sz], w_o_h0[:, :],
                         h0T[:, lo:lo + sz], start=False, stop=True)
    oT = work.tile([64, N], F32, tag="oT")
    nc.scalar.activation(oT[:, :], pg3[0:64, :], AF.Tanh,
                         bias=cols["b_o"][:, 0:1])

    o_sb = work.tile([128, NCH * D], F32, tag="o_sb")
    for c in range(NCH):
        ptr = ps_tr.tile([128, 384], F32, tag="tr")
        nc.tensor.transpose(ptr[:, 0:64], oT[:, c * 128:(c + 1) * 128],
                            ident[0:64, 0:64])
        nc.scalar.copy(o_sb[:, c * D:(c + 1) * D], ptr[:, 0:64])
    nc.sync.dma_start(d["out"][it].rearrange("(c p) d -> p c d", p=128),
                      o_sb[:, :])


_CACHE = {}


def _get_program():
    if "nc" not in _CACHE:
        _CACHE["nc"] = _build_program()
    return _CACHE["nc"]


def _get_runner():
    """Cached jitted SPMD runner (mirrors bass2jax.run_bass_via_pjrt but
    reuses one jax.jit across calls)."""
    if "runner" in _CACHE:
        return _CACHE["runner"]
    import jax
    from jax.experimental.shard_map import shard_map
    from jax.sharding import Mesh, PartitionSpec
    from concourse import bass2jax, mybir as mb

    nc = _get_program()
    bass2jax.install_neuronx_cc_hook()
    part_name = (nc.partition_id_tensor.name
                 if nc.partition_id_tensor is not None else None)
    in_names, out_names, out_avals, zero_shapes = [], [], [], []
    for alloc in nc.m.functions[0].allocations:
        if not isinstance(alloc, mb.MemoryLocationSet):
            continue
        name = alloc.memorylocations[0].name
        if alloc.kind == "ExternalInput":
            if name != part_name:
                in_names.append(name)
        elif alloc.kind == "ExternalOutput":
            shape = tuple(alloc.tensor_shape)
            dtype = mb.dt.np(alloc.dtype)
            out_names.append(name)
            out_avals.append(jax.core.ShapedArray(shape, dtype))
            zero_shapes.append((shape, dtype))
    n_params = len(in_names)
    all_names = in_names + out_names
    if part_name is not None:
        all_names = all_names + [part_name]
    donate = tuple(range(n_params, n_params + len(out_names)))

    def _body(*args):
        operands = list(args)
        if part_name is not None:
            operands.append(bass2jax.partition_id_tensor())
        outs = bass2jax._bass_exec_p.bind(
            *operands,
            out_avals=tuple(out_avals),
            in_names=tuple(all_names),
            out_names=tuple(out_names),
            lowering_input_output_aliases=(),
            sim_require_finite=True,
            sim_require_nnan=True,
            nc=nc,
        )
        return tuple(outs)

    devices = jax.devices()[:N_CORES]
    mesh = Mesh(np.asarray(devices), ("core",))
    n_all = n_params + len(out_names)
    sharded = jax.jit(
        shard_map(_body, mesh=mesh,
                  in_specs=(PartitionSpec("core"),) * n_all,
                  out_specs=(PartitionSpec("core"),) * len(out_names),
                  check_rep=False),
        donate_argnums=donate, keep_unused=True,
    )
    _CACHE["runner"] = (sharded, in_names, out_names, zero_shapes)
    return _CACHE["runner"]


def _concat_inputs(inputs):
    """Build the global concatenated input list (order = in_names)."""
    sharded, in_names, out_names, zero_shapes = _get_runner()
    A = np.ascontiguousarray(np.asarray(inputs["A"], dtype=np.float32))
    prop = np.ascontiguousarray(np.asarray(inputs["prop_state"],
                                           dtype=np.float32))
    per_core = {
        "A": A.reshape(N_CORES, PER_CORE, N, 2 * NE),
        "prop_state": prop.reshape(N_CORES, PER_CORE, N, D),
    }
    shared = {nm: np.ascontiguousarray(np.asarray(inputs[nm],
                                                  dtype=np.float32))
              for nm in W_NAMES}
    concat = []
    for name in in_names:
        if name in per_core:
            v = per_core[name]
            concat.append(v.reshape(N_CORES * v.shape[1], *v.shape[2:]))
        else:
            v = shared[name]
            concat.append(np.concatenate([v] * N_CORES, axis=0))
    return concat


def _run_concat(concat):
    sharded, in_names, out_names, zero_shapes = _get_runner()
    zeros = [np.zeros((N_CORES * s[0], *s[1:]), dt) for s, dt in zero_shapes]
    out_arrs = sharded(*concat, *zeros)
    return {name: out_arrs[i] for i, name in enumerate(out_names)}


def kernel(**inputs):
    concat = _concat_inputs(inputs)
    outs = _run_concat(concat)
    out = np.asarray(outs["out"])
    return out.reshape(B, N, D).astype(np.float32)


if __name__ == "__main__":
    rng = np.random.default_rng(0)
    inputs = {
        "prop_state": rng.standard_normal((B, N, D), dtype=np.float32),
        "annotation": rng.standard_normal((B, N, D), dtype=np.float32),
        "A": rng.random((B, N, 2 * NE), dtype=np.float32),
        "node_mask": np.ones((B, N), dtype=bool),
        "W_in": rng.standard_normal((E, D, D), dtype=np.float32) * 0.05,
        "b_in": rng.standard_normal((E, D), dtype=np.float32) * 0.05,
        "W_out": rng.standard_normal((E, D, D), dtype=np.float32) * 0.05,
        "b_out": rng.standard_normal((E, D), dtype=np.float32) * 0.05,
        "W_r": rng.standard_normal((D, 3 * D), dtype=np.float32) * 0.05,
        "b_r": rng.standard_normal((D,), dtype=np.float32) * 0.05,
        "W_z": rng.standard_normal((D, 3 * D), dtype=np.float32) * 0.05,
        "b_z": rng.standard_normal((D,), dtype=np.float32) * 0.05,
        "W_h": rng.standard_normal((D, 3 * D), dtype=np.float32) * 0.05,
        "b_h": rng.standard_normal((D,), dtype=np.float32) * 0.05,
        "W_o": rng.standard_normal((D, 2 * D), dtype=np.float32) * 0.05,
        "b_o": rng.standard_normal((D,), dtype=np.float32) * 0.05,
    }
    out = kernel(**inputs)
    print("out", out.shape, out.dtype, float(np.abs(out).max()))
